# revision 33
# baseline (speedup 1.0000x reference)
"""Causal multi-head attention with RoPE on 8 Trainium2 NeuronCores.

Problem: B=2, N=2048, DIM=1024, H=16, DH=64, fp32.
Sharding: batch x head parallel - core c owns batch c//4 and heads
4*(c%4)..4*(c%4)+4 (columns of Wq/Wk/Wv, rows of Wo). Each core computes
its partial output projection [DIM, N] for its batch; the host sums the
4 partials per batch (the "all-reduce") and adds the bias.

On-device pipeline (matmul operands float32r = full PE column rate):
  p1(chunk): QKV projections (K=1024 accum) per 128-feature block (2
             heads), RoPE fused from PSUM on DVE/Pool -> qt/kt resident
             [128, 2, N]; V transposed via PE into [j, d] layout with a
             ones column for softmax sums.
  p2(ch):    per head-group hg (2 heads sharing a partition tile): per
             j-block, both heads' S^T = k q^T as two K=64 matmuls on
             disjoint PE row groups (concurrent); exp on ACT -> f32r;
             causal mask applied post-exp as a 0/1 multiply on the
             128-col diagonal slab (DVE); diagonal blocks are
             column-restricted to >=256 cols; augmented V-matmul
             accumulates O'^T + softmax sums; fast-reciprocal normalize;
             fused Wo projection (K=256 over both head groups).
p1 emission is a generator; p2 consumes it between j-blocks so QKV
matmuls fill the PE while ACT computes exps.
"""
import numpy as np
import bass_rust
from concourse import bacc
import concourse.mybir as mybir
from concourse.tile import TileContext
from concourse.bass_utils import run_bass_kernel_spmd

B, N, DIM, H, DH = 2, 2048, 1024, 16, 64
NCORES = 8
HPC = 4                    # heads per core
NHG = 2                    # head groups (2 heads = 128 partitions each)
CHUNK = 512
NCH = N // CHUNK           # 4 token chunks per core
NCB = DIM // 128           # 8 contraction blocks for QKV
NJB = N // 128             # 16 j-blocks

F32 = mybir.dt.float32
F32R = mybir.dt.float32r
BF16 = mybir.dt.bfloat16

RECIP_FAST = True
POOL_ADD = False
POOL_MASK = False
ROT_W = True
DEBUG = False

_NC_CACHE = {}


def build(reps=1):
    nc = bacc.Bacc()
    xTD = nc.dram_tensor("xT", [DIM, N], BF16, kind="ExternalInput")
    wqD = nc.dram_tensor("wq", [DIM, 256], BF16, kind="ExternalInput")
    wkD = nc.dram_tensor("wk", [DIM, 256], BF16, kind="ExternalInput")
    wvD = nc.dram_tensor("wv", [DIM, 256], BF16, kind="ExternalInput")
    wqRD = nc.dram_tensor("wqR", [DIM, 256], BF16, kind="ExternalInput")
    wkRD = nc.dram_tensor("wkR", [DIM, 256], BF16, kind="ExternalInput")
    woD = nc.dram_tensor("wo", [256, DIM], F32R, kind="ExternalInput")
    cosD = nc.dram_tensor("cosT", [DH, N], F32, kind="ExternalInput")
    sinsD = nc.dram_tensor("sinsT", [DH, N], F32, kind="ExternalInput")
    ident2D = nc.dram_tensor("ident2D", [128, DH], F32R, kind="ExternalInput")
    masksD = nc.dram_tensor("masksD", [128, 2, 384], F32R, kind="ExternalInput")
    onesColD = nc.dram_tensor("onesColD", [128, HPC, NJB, 1], F32R,
                              kind="ExternalInput")
    outD = nc.dram_tensor("outT", [DIM, N], F32, kind="ExternalOutput")
    if DEBUG:
        dbgqD = nc.dram_tensor("dbgq", [128, NHG, N], F32, kind="ExternalOutput")
        dbgkD = nc.dram_tensor("dbgk", [128, NHG, N], F32, kind="ExternalOutput")
        dbgvD = nc.dram_tensor("dbgv", [128, HPC, NJB, DH + 1], F32,
                               kind="ExternalOutput")
        dbgeD = nc.dram_tensor("dbge", [128, 2, CHUNK], F32, kind="ExternalOutput")
        dbgpD = nc.dram_tensor("dbgp", [DH + 1, 2, CHUNK], F32,
                               kind="ExternalOutput")
        dbgrD = nc.dram_tensor("dbgr", [DH, 2, CHUNK], F32,
                               kind="ExternalOutput")
        dbgoD = nc.dram_tensor("dbgo", [128, 2, CHUNK], F32,
                               kind="ExternalOutput")

    Exp = mybir.ActivationFunctionType.Exp
    Copy = mybir.ActivationFunctionType.Copy

    with TileContext(nc) as tc:
        with (
            tc.tile_pool(name="const", bufs=1) as cp,
            tc.tile_pool(name="sb", bufs=2) as sb,
            tc.tile_pool(name="ps", bufs=1, space="PSUM") as ps,
        ):
            ident2 = cp.tile([128, DH], F32R)
            masks = cp.tile([128, 2, 384], F32R)
            wq = cp.tile([128, NCB, 2, 128], BF16)
            wk = cp.tile([128, NCB, 2, 128], BF16)
            wv = cp.tile([128, NCB, 2, 128], BF16)
            wo = cp.tile([128, 2, NCB, 128], F32R)
            cos2 = cp.tile([128, N], F32)
            sins2 = cp.tile([128, N], F32)
            qt = cp.tile([128, NHG, N], F32R)
            kt = cp.tile([128, NHG, N], F32R)
            v_aug = cp.tile([128, HPC, NJB, DH + 1], F32R)

            wqR = cp.tile([128, NCB, 2, 128], BF16)
            wkR = cp.tile([128, NCB, 2, 128], BF16)

            # front-load: wq cb0 + chunk-0 x cb0 first so the PE starts early
            def wload(t, d, cbs):
                nc.sync.dma_start(
                    out=t[:, cbs, :, :],
                    in_=d[cbs.start * 128:cbs.stop * 128, :].rearrange(
                        "(cb p) (f m) -> p cb f m", p=128, m=128))
            xt0 = sb.tile([128, NCB, CHUNK], BF16, tag="xt", bufs=2,
                          name="xt_first")
            wload(wq, wqD, slice(0, 1))
            nc.sync.dma_start(
                out=xt0[:, 0:1, :],
                in_=xTD[0:128, 0:CHUNK].rearrange("(cb p) n -> p cb n", p=128))
            wload(wqR, wqRD, slice(0, 1))
            wload(wq, wqD, slice(1, NCB))
            wload(wqR, wqRD, slice(1, NCB))
            # bulk x + k weights + rope tables ride the second HWDGE queue
            # (ACT) so the q weights don't starve behind them
            for cbs in (slice(1, 4), slice(4, NCB)):
                nc.scalar.dma_start(
                    out=xt0[:, cbs, :],
                    in_=xTD[cbs.start * 128:cbs.stop * 128, 0:CHUNK].rearrange(
                        "(cb p) n -> p cb n", p=128))
            nc.scalar.dma_start(out=cos2[0:64, :], in_=cosD[:])
            nc.scalar.dma_start(out=sins2[0:64, :], in_=sinsD[:])
            for t, d in ((wk, wkD), (wkR, wkRD)):
                nc.scalar.dma_start(
                    out=t, in_=d[:].rearrange("(cb p) (f m) -> p cb f m",
                                              p=128, m=128))
            for t in (cos2, sins2):
                nc.scalar.dma_start(out=t[64:128, :], in_=t[0:64, :])

            # v/wo + small constants ride the SWDGE queue
            nc.gpsimd.dma_start(
                out=wv, in_=wvD[:].rearrange("(cb p) (f m) -> p cb f m",
                                             p=128, m=128))
            nc.gpsimd.dma_start(out=ident2, in_=ident2D[:])
            nc.gpsimd.dma_start(out=v_aug[:, :, :, DH:DH + 1], in_=onesColD[:])
            nc.gpsimd.dma_start(out=masks, in_=masksD[:])
            nc.gpsimd.dma_start(
                out=wo, in_=woD[:].rearrange("(kb p) (db m) -> p kb db m",
                                             p=128, m=128))

            def p1_gen(ch):
                """QKV + RoPE + V transpose for one 512-token chunk.
                Yields after each PE instruction group."""
                t0 = ch * CHUNK
                if ch == 0:
                    xt = xt0
                else:
                    xt = sb.tile([128, NCB, CHUNK], BF16, tag="xt", bufs=2,
                                 name=f"xt{ch}")
                    for hb in range(4):
                        nc.sync.dma_start(
                            out=xt[:, hb * 2:(hb + 1) * 2, :],
                            in_=xTD[hb * 2 * 128:(hb + 1) * 2 * 128,
                                    t0:t0 + CHUNK].rearrange(
                                "(cb p) n -> p cb n", p=128))
                csl = slice(t0, t0 + CHUNK)
                for which, W, WR in (("q", wq, wqR), ("k", wk, wkR)):
                    dst = qt if which == "q" else kt
                    for fb in range(2):
                        tmp = sb.tile([128, CHUNK], F32, tag="tmp", bufs=2,
                                      name=f"t{ch}{which}{fb}")
                        tmp2 = sb.tile([128, CHUNK], F32, tag="tmp2", bufs=2,
                                       name=f"u{ch}{which}{fb}")
                        if ROT_W:
                            # rope from two PSUM accumulators; the rotated
                            # projection comes from host-rotated weights
                            pp = ps.tile([128, CHUNK], F32, tag="sm", bufs=2,
                                         name=f"pp{ch}{which}{fb}")
                            ppR = ps.tile([128, CHUNK], F32, tag="sm", bufs=2,
                                          name=f"pr{ch}{which}{fb}")
                            for cb in range(NCB):
                                nc.tensor.matmul(pp, W[:, cb, fb, :],
                                                 xt[:, cb, :],
                                                 start=(cb == 0),
                                                 stop=(cb == NCB - 1))
                                nc.tensor.matmul(ppR, WR[:, cb, fb, :],
                                                 xt[:, cb, :],
                                                 start=(cb == 0),
                                                 stop=(cb == NCB - 1))
                                yield
                            nc.vector.tensor_mul(tmp, pp, cos2[:, csl])
                            nc.vector.tensor_mul(tmp2, ppR, sins2[:, csl])
                        else:
                            pp = ps.tile([128, CHUNK], F32, tag="sm", bufs=2,
                                         name=f"pp{ch}{which}{fb}")
                            for cb in range(NCB):
                                nc.tensor.matmul(pp, W[:, cb, fb, :],
                                                 xt[:, cb, :],
                                                 start=(cb == 0),
                                                 stop=(cb == NCB - 1))
                                yield
                            raw = sb.tile([128, CHUNK], F32, tag="tmp", bufs=2,
                                          name=f"w{ch}{which}{fb}")
                            nc.vector.tensor_copy(raw, pp)
                            rawsw = sb.tile([128, CHUNK], F32, tag="tmp2",
                                            bufs=2, name=f"x{ch}{which}{fb}")
                            for hh in (0, 64):
                                a, bnd, c2 = hh, hh + 32, hh + 64
                                nc.gpsimd.dma_start(out=rawsw[a:bnd, :],
                                                    in_=raw[bnd:c2, :])
                                nc.gpsimd.dma_start(out=rawsw[bnd:c2, :],
                                                    in_=raw[a:bnd, :])
                            nc.vector.tensor_mul(tmp, raw, cos2[:, csl])
                            nc.vector.tensor_mul(tmp2, rawsw, sins2[:, csl])
                        if POOL_ADD:
                            nc.gpsimd.tensor_add(dst[:, fb, csl], tmp, tmp2)
                        else:
                            nc.vector.tensor_add(dst[:, fb, csl], tmp, tmp2)
                for fb in range(2):
                    pp = ps.tile([128, CHUNK], F32, tag="sm", bufs=2,
                                 name=f"pp{ch}v{fb}")
                    for cb in range(NCB):
                        nc.tensor.matmul(pp, wv[:, cb, fb, :], xt[:, cb, :],
                                         start=(cb == 0), stop=(cb == NCB - 1))
                        yield
                    vtc = sb.tile([128, CHUNK], F32, tag="vtc", bufs=2,
                                  name=f"v{ch}{fb}")
                    nc.vector.tensor_copy(vtc, pp)
                    for tb in range(4):
                        jb = ch * 4 + tb
                        pts = []
                        for h in range(2):
                            pt = ps.tile([128, DH], F32, tag="sm",
                                         bufs=2, name=f"pt{ch}{fb}{tb}{h}")
                            nc.tensor.transpose(
                                pt,
                                vtc[h * 64:(h + 1) * 64,
                                    tb * 128:(tb + 1) * 128],
                                ident2[h * 64:(h + 1) * 64, :].bitcast(F32))
                            pts.append(pt)
                        yield
                        for h in range(2):
                            nc.vector.tensor_copy(
                                v_aug[:, fb * 2 + h, jb, 0:DH], pts[h])

            class Feeder:
                def __init__(self):
                    self.gens = []

                def add(self, gen, defer=False):
                    self.gens.append([gen, defer])

                def feed(self, n):
                    while n > 0 and self.gens:
                        try:
                            next(self.gens[0][0])
                            n -= 1
                        except StopIteration:
                            self.gens.pop(0)

                def drain(self):
                    # flush non-deferred gens only (deferred ones are tail
                    # filler for the final chunk)
                    while self.gens and not self.gens[0][1]:
                        try:
                            next(self.gens[0][0])
                        except StopIteration:
                            self.gens.pop(0)

                def drain_all(self):
                    while self.gens:
                        self.feed(1 << 30)

            def p2_chunk(ch, feeder, osb_eng):
                """Attention + projection for i-chunk ch (all 4 heads)."""
                gcol = ch * CHUNK
                njb = 4 * (ch + 1)
                ot = sb.tile([128, 2, CHUNK], F32R, tag="ot", bufs=2,
                             name=f"ot{ch}")
                for hg in range(NHG):
                    pos = [ps.tile([DH + 1, CHUNK], F32, tag="po", bufs=2,
                                   name=f"po{ch}{hg}{h}") for h in range(2)]
                    for jb in range(njb):
                        r = jb - 4 * ch
                        c0 = 0 if r < 0 else min(r * 128, 256)
                        w = CHUNK - c0
                        jc = jb * 128
                        pst = ps.tile([128, 2, CHUNK], F32, tag="pst", bufs=2,
                                      name=f"ps{ch}{hg}{jb}")
                        for h in range(2):
                            qr = slice(h * 64, (h + 1) * 64)
                            nc.tensor.matmul(
                                pst[:, h, c0:CHUNK], kt[qr, hg, jc:jc + 128],
                                qt[qr, hg, gcol + c0:gcol + CHUNK],
                                start=True, stop=True)
                        feeder.feed(3)
                        expt = sb.tile([128, 2, CHUNK], F32R, tag="expt",
                                       bufs=3, name=f"e{ch}{hg}{jb}")
                        nc.scalar.activation(expt[:, :, 0:w],
                                             pst[:, :, c0:CHUNK], Exp)
                        if r >= 0:
                            # 0/1 causal mask on the diagonal slab
                            msl = (slice(0, 128) if r < 3
                                   else slice(128, 384))
                            mw = msl.stop - msl.start
                            eng = nc.gpsimd if POOL_MASK else nc.vector
                            eng.tensor_mul(expt[:, :, 0:mw],
                                           expt[:, :, 0:mw],
                                           masks[:, :, msl])
                        if DEBUG and ch == 0 and hg == 0 and jb == 0:
                            nc.sync.dma_start(out=dbgeD[:], in_=expt.bitcast(F32))
                        for h in range(2):
                            nc.tensor.matmul(
                                pos[h][:, c0:CHUNK],
                                v_aug[:, hg * 2 + h, jb, :],
                                expt[:, h, 0:w],
                                start=(jb == 0), stop=(jb == njb - 1))
                        feeder.feed(1)
                    if DEBUG and ch == 0 and hg == 0:
                        pod = sb.tile([DH + 1, 2, CHUNK], F32, tag="pod",
                                      bufs=1, name="pod")
                        for h in range(2):
                            nc.vector.tensor_copy(pod[:, h, :], pos[h])
                        nc.sync.dma_start(out=dbgpD[:], in_=pod)
                    for h in range(2):
                        rrow = sb.tile([1, CHUNK], F32, tag="rrow", bufs=4,
                                       name=f"r{ch}{hg}{h}")
                        srow = sb.tile([1, CHUNK], F32, tag="srow",
                                       bufs=4, name=f"s{ch}{hg}{h}")
                        nc.scalar.activation(srow, pos[h][DH:DH + 1, :], Copy)
                        if RECIP_FAST:
                            nc.vector.reciprocal_approx_fast(rrow, srow)
                        else:
                            nc.vector.reciprocal(rrow, srow)
                        rb = sb.tile([DH, CHUNK], F32, tag="rb", bufs=2,
                                     name=f"rb{ch}{hg}{h}")
                        nc.gpsimd.partition_broadcast(rb, rrow)
                        nc.vector.tensor_mul(
                            ot[h * 64:(h + 1) * 64, hg, :],
                            pos[h][0:DH, :], rb)
                        if DEBUG and ch == 0 and hg == 0:
                            nc.sync.dma_start(out=dbgrD[:, h, :], in_=rb)
                    feeder.feed(4)
                if DEBUG and ch == 0:
                    nc.sync.dma_start(out=dbgoD[:], in_=ot.bitcast(F32))

                # fused output projection, K=256 over both head groups
                def outproj():
                    for db in range(NCB):
                        ppr = ps.tile([128, CHUNK], F32, tag="sm", bufs=2,
                                      name=f"pj{ch}{db}")
                        nc.tensor.matmul(ppr, wo[:, 0, db, :], ot[:, 0, :],
                                         start=True, stop=False)
                        nc.tensor.matmul(ppr, wo[:, 1, db, :], ot[:, 1, :],
                                         start=False, stop=True)
                        yield
                        osb = sb.tile([128, CHUNK], F32, tag="osb", bufs=3,
                                      name=f"o{ch}{db}")
                        if osb_eng[db % 2] == "v":
                            nc.vector.tensor_copy(osb, ppr)
                        else:
                            nc.scalar.activation(osb, ppr, Copy)
                        dma_eng = nc.sync if db % 2 == 0 else nc.gpsimd
                        dma_eng.dma_start(
                            out=outD[db * 128:(db + 1) * 128,
                                     gcol:gcol + CHUNK],
                            in_=osb)
                if ch == NCH - 2:
                    # defer this chunk's projection into the final chunk's
                    # attention loop, where the p1 feeder has run dry
                    feeder.add(outproj(), defer=True)
                else:
                    g = outproj()
                    for _ in g:
                        feeder.feed(2)

            for _ in range(reps):
                feeder = Feeder()
                feeder.add(p1_gen(0))
                feeder.drain()
                for ch in range(NCH):
                    if ch + 1 < NCH:
                        feeder.add(p1_gen(ch + 1))
                    p2_chunk(ch, feeder, ("v", "s"))
                    # p1(ch+1) must be fully emitted before p2(ch+1)'s S
                    # matmuls enter the PE queue (in-order engine).
                    feeder.drain()
                feeder.drain_all()
                if DEBUG:
                    nc.sync.dma_start(out=dbgqD[:], in_=qt.bitcast(F32))
                    nc.sync.dma_start(out=dbgkD[:], in_=kt.bitcast(F32))
                    nc.sync.dma_start(out=dbgvD[:], in_=v_aug.bitcast(F32))
    nc.compile()
    return nc


def _get_nc(reps=1):
    if reps not in _NC_CACHE:
        _NC_CACHE[reps] = build(reps)
    return _NC_CACHE[reps]


def make_in_maps(x, pos_emb, Wq, Wk, Wv, Wo):
    x = np.asarray(x, np.float32)
    pos_emb = np.asarray(pos_emb, np.float32)
    Wq = np.asarray(Wq, np.float32)
    Wk = np.asarray(Wk, np.float32)
    Wv = np.asarray(Wv, np.float32)
    Wo = np.asarray(Wo, np.float32)

    cosT = np.ascontiguousarray(np.cos(pos_emb).T)          # [DH, N]
    sinsT = np.ascontiguousarray(np.sin(pos_emb).T)         # [DH, N]
    scale = np.float32(DH ** -0.5)

    ident2 = np.tile(np.eye(DH, dtype=np.float32), (2, 1))
    # masks: [T128 | zeros128 | T128] triangular 0/1, duplicated per head
    jj = np.arange(128)[:, None]
    mm = np.arange(128)[None, :]
    tri = (jj <= mm).astype(np.float32)
    m384 = np.concatenate([tri, np.zeros((128, 128), np.float32), tri], 1)
    masks = np.broadcast_to(m384[:, None, :], (128, 2, 384)).copy()
    ones_col = np.ones((128, HPC, NJB, 1), np.float32)

    def rot_cols(W):
        # WR = rotate_half applied to the output features of W (per 64-dh
        # head block): WR[:, 0:32] = -W[:, 32:64], WR[:, 32:64] = W[:, 0:32]
        R = W.reshape(DIM, H, 2, 32)
        return np.concatenate([-R[:, :, 1:2], R[:, :, 0:1]], 2).reshape(DIM, H * DH)

    WqR, WkR = rot_cols(Wq), rot_cols(Wk)
    in_maps = []
    for c in range(NCORES):
        b = c // 4
        cols = slice((c % 4) * 256, (c % 4 + 1) * 256)
        import ml_dtypes
        bf = ml_dtypes.bfloat16
        xT = np.ascontiguousarray(x[b].T).astype(bf)        # [DIM, N]
        in_maps.append(dict(
            xT=xT,
            wq=(np.ascontiguousarray(Wq[:, cols]) * scale).astype(bf),
            wk=np.ascontiguousarray(Wk[:, cols]).astype(bf),
            wv=np.ascontiguousarray(Wv[:, cols]).astype(bf),
            wqR=(np.ascontiguousarray(WqR[:, cols]) * scale).astype(bf),
            wkR=np.ascontiguousarray(WkR[:, cols]).astype(bf),
            wo=np.ascontiguousarray(Wo[cols, :]),
            cosT=cosT, sinsT=sinsT, ident2D=ident2,
            masksD=masks, onesColD=ones_col,
        ))
    return in_maps


def run(in_maps, trace=False, reps=1, **kw):
    nc = _get_nc(reps)
    return run_bass_kernel_spmd(nc, in_maps, list(range(NCORES)),
                                trace=trace, **kw)


def kernel(x, pos_emb, Wq, Wk, Wv, Wo, bo):
    in_maps = make_in_maps(x, pos_emb, Wq, Wk, Wv, Wo)
    res = run(in_maps)
    out = np.zeros((B, N, DIM), np.float64)
    for c in range(NCORES):
        out[c // 4] += res.results[c]["outT"].T
    out += np.asarray(bo, np.float32)[None, None, :]
    return out.astype(np.float32)
